# revision 14
# baseline (speedup 1.0000x reference)
"""BERT self-attention (B=8, S=1024, D=768, H=12) on 8 TRN2 NeuronCores.

Sharding: batch across the 8 cores (one batch element per core).

Per-core dataflow (all matmuls fp32r on the tensor engine):
  - host pre-transposes hs[b] -> hsT [D, S] and the weights -> W.T [D, D]
    so the contraction dim (din) lands on SBUF partitions.
  - qT[dout, s], k'T[dout, s] = W.T-tiles (stationary) x hsT (moving);
    k'T additionally folds the 1/sqrt(dh) scale (host, into Wk/bk) and the
    multiplicative click_times[ks] gate (on-chip, per-column multiply).
  - v[s, dout] = hsT-tiles (stationary) x Wv.T (moving), stored head-major
    [s, (h, 65)] with a ones column per head; rows scaled by exp(mask[ks])
    (folds the additive attention mask through the exp).
  - per head: scoresT[ks, qs] = kT_head.T @ qT_head (K=64; adjacent heads
    packed concurrently via PE row tiling), expT = Exp(scoresT) straight
    out of PSUM on the scalar engine, ctxT[65, qs] accumulates
    v_aug.T @ expT over ks; row 64 is the softmax denominator.
  - normalize: partition-broadcast of the denominator row + reciprocal +
    multiply; DMA ctxT per head; host transposes back on unshard.
"""

import sys

sys.path.insert(0, "/opt/trn_rl_repo")

import numpy as np

B, S, D, H = 8, 1024, 768, 12
DH = D // H  # 64
NT = D // 128  # 6 dout/din tiles
NS = S // 128  # 8 s tiles
QS = 512  # qs chunk (fp32 moving-operand max)

_built = None


def _apply_workarounds():
    """Container fixes: (1) walrus here accepts at most one sync wait on the
    Tile tail Drain -> split extra waits onto SP nops; (2) antenv.axon_hooks
    is missing from the image (needed only for trace=True profiling)."""
    import os

    import concourse.tile as tile
    from concourse.vector_clock import ScopedClock

    if getattr(tile.TileContext, "_drain_split_patched", False):
        return

    def _drain_and_barrier(self, tick_clock, wait_clock):
        drain_inst = self.nc.sync.drain()
        wait_clock.add_sem_waits(
            drain_inst.ins, ScopedClock({None: tick_clock.global_clock})
        )
        si = drain_inst.ins.sync_info
        if si is not None and len(si.on_wait) > 1:
            waits = list(si.on_wait)
            si.on_wait = waits[:1]
            for w in waits[1:]:
                nop = self.nc.sync.nop(nofuse=True, hint="drain_wait_split")
                nsi = nop.ins.sync_info
                if nsi is None:
                    import bass_rust

                    nop.ins.sync_info = bass_rust.SyncInfo(on_update=[], on_wait=[w])
                else:
                    nsi.on_wait = [w]

        self.nc.all_engine_barrier()
        assert self.sems is not None
        popped = self.nc._tile_sem_poison_stack.pop()
        assert popped is self._sem_poison
        self.nc.clear_and_free_semaphores(list(self.sems.allocated().values()))
        self.nc.all_engine_barrier()

    tile.TileContext._drain_and_barrier = _drain_and_barrier
    tile.TileContext._drain_split_patched = True

    hooks_src = (
        "_axon_ntff_profile_hook = None\n\n\n"
        "def set_axon_ntff_profile_hook(hook):\n"
        "    global _axon_ntff_profile_hook\n"
        "    _axon_ntff_profile_hook = hook\n\n\n"
        "def get_axon_ntff_profile_hook():\n"
        "    return _axon_ntff_profile_hook\n"
    )
    for d in ("/root/.axon_site/_ro/trn_rl_repo/antenv", "/opt/trn_rl_repo/antenv"):
        path = os.path.join(d, "axon_hooks.py")
        try:
            if os.path.isdir(d) and not os.path.exists(path):
                with open(path, "w") as f:
                    f.write(hooks_src)
        except OSError:
            pass


def _build(debug=False):
    import concourse.bass as bass
    import concourse.tile as tile
    from concourse import mybir

    f32 = mybir.dt.float32
    f32r = mybir.dt.float32r
    Exp = mybir.ActivationFunctionType.Exp
    mult = mybir.AluOpType.mult

    nc = bass.Bass()
    hsT_d = nc.dram_tensor("hsT", [D, S], f32r, kind="ExternalInput")
    wT_d = {
        w: nc.dram_tensor(f"w{w}T", [D, D], f32r, kind="ExternalInput")
        for w in ("q", "k", "v")
    }
    bqT_d = nc.dram_tensor("bqT", [128, NT], f32, kind="ExternalInput")
    bkT_d = nc.dram_tensor("bkT", [128, NT], f32, kind="ExternalInput")
    bvB_d = nc.dram_tensor("bvB", [128, D], f32, kind="ExternalInput")
    clickB_d = nc.dram_tensor("clickB", [128, S], f32, kind="ExternalInput")
    maskT_d = nc.dram_tensor("maskT", [128, NS], f32, kind="ExternalInput")
    ones64_d = nc.dram_tensor("ones64", [1, DH], f32r, kind="ExternalInput")
    vones_d = nc.dram_tensor("vones", [128, NS, H], f32r, kind="ExternalInput")
    out_d = nc.dram_tensor("out", [H, DH, S], f32, kind="ExternalOutput")
    if debug:
        qT_dbg = nc.dram_tensor("qTd", [128, NT, S], f32, kind="ExternalOutput")
        kT_dbg = nc.dram_tensor("kTd", [128, NT, S], f32, kind="ExternalOutput")
        v_dbg = nc.dram_tensor("vd", [128, NS, H * (DH + 1)], f32, kind="ExternalOutput")
        exp_dbg = nc.dram_tensor("expd", [128, S], f32, kind="ExternalOutput")
        ctxraw_dbg = nc.dram_tensor("ctxrawd", [DH + 1, S], f32, kind="ExternalOutput")

    with tile.TileContext(nc) as tc:
        from contextlib import ExitStack

        with ExitStack() as ctx:
            consts = ctx.enter_context(tc.tile_pool(name="consts", bufs=1))
            big = ctx.enter_context(tc.tile_pool(name="big", bufs=1))
            exps = ctx.enter_context(tc.tile_pool(name="exps", bufs=3))
            fin = ctx.enter_context(tc.tile_pool(name="fin", bufs=2))
            pp = ctx.enter_context(tc.tile_pool(name="pp", bufs=2, space="PSUM"))
            psc = ctx.enter_context(tc.tile_pool(name="psc", bufs=1, space="PSUM"))
            pcx = ctx.enter_context(tc.tile_pool(name="pcx", bufs=2, space="PSUM"))

            # ---- constants + full-tensor loads ----
            bqT = consts.tile([128, NT], f32)
            nc.sync.dma_start(out=bqT, in_=bqT_d[:])
            bkT = consts.tile([128, NT], f32)
            nc.sync.dma_start(out=bkT, in_=bkT_d[:])
            bvB = consts.tile([128, D], f32)
            nc.sync.dma_start(out=bvB, in_=bvB_d[:])
            clickB = consts.tile([128, S], f32)
            nc.sync.dma_start(out=clickB, in_=clickB_d[:])
            maskT = consts.tile([128, NS], f32)
            nc.sync.dma_start(out=maskT, in_=maskT_d[:])

            hsT = big.tile([128, NT, S], f32r)
            nc.sync.dma_start(out=hsT, in_=hsT_d.rearrange("(t p) s -> p t s", p=128))
            wT = {}
            for w in ("q", "k", "v"):
                wT[w] = big.tile([128, NT, D], f32r, tag=f"w{w}", name=f"w{w}sb")
                nc.sync.dma_start(
                    out=wT[w], in_=wT_d[w].rearrange("(t p) d -> p t d", p=128)
                )

            ones64 = consts.tile([1, DH], f32r)
            nc.sync.dma_start(out=ones64, in_=ones64_d[:])

            qT = big.tile([128, NT, S], f32r, tag="qT")
            kT = big.tile([128, NT, S], f32r, tag="kT")
            # v_aug: [s_partition, s_tile, head-major (h, dh | ones)]
            v = big.tile([128, NS, H * (DH + 1)], f32r, tag="v")

            def proj_qk(t):
                """qT/k'T for dout-tile t (heads 2t, 2t+1): emit both W's."""
                for w, dest, bias in (("q", qT, bqT), ("k", kT, bkT)):
                    for c in range(S // QS):
                        cs = slice(c * QS, (c + 1) * QS)
                        ps = pp.tile([128, QS], f32, tag="proj")
                        for k in range(NT):
                            nc.tensor.matmul(
                                ps,
                                wT[w][:, k, t * 128 : (t + 1) * 128],
                                hsT[:, k, cs],
                                start=(k == 0),
                                stop=(k == NT - 1),
                            )
                        nc.vector.tensor_scalar_add(
                            dest[:, t, cs], ps, bias[:, t : t + 1]
                        )
                        if w == "k":
                            nc.vector.tensor_tensor(
                                out=dest[:, t, cs],
                                in0=dest[:, t, cs],
                                in1=clickB[:, cs],
                                op=mult,
                            )

            def proj_v(si):
                """v rows for s-tile si, head-major with ones col, mask-scaled."""
                vsi = v[:, si, :].rearrange("p (h e) -> p h e", e=DH + 1)
                for c0, cn in ((0, 512), (512, 256)):
                    h0, nh = c0 // DH, cn // DH
                    ps = pp.tile([128, cn], f32, tag="proj")
                    for k in range(NT):
                        nc.tensor.matmul(
                            ps,
                            hsT[:, k, si * 128 : (si + 1) * 128],
                            wT["v"][:, k, c0 : c0 + cn],
                            start=(k == 0),
                            stop=(k == NT - 1),
                        )
                    nc.vector.tensor_tensor(
                        out=vsi[:, h0 : h0 + nh, 0:DH],
                        in0=ps.rearrange("p (h e) -> p h e", e=DH),
                        in1=bvB[:, c0 : c0 + cn].rearrange("p (h e) -> p h e", e=DH),
                        op=mybir.AluOpType.add,
                    )
                nc.sync.dma_start(
                    out=vsi[:, :, DH : DH + 1], in_=vones_d[:, si, :]
                )
                nc.vector.tensor_scalar_mul(v[:, si, :], v[:, si, :], maskT[:, si : si + 1])

            def attn_pair(t, filler):
                """Heads a=2t (partitions 0:64 of tile t) and b=2t+1 (64:128).

                filler(j) emits gap-filling PE work before iteration j's
                PSUM-recycle stall."""
                ctx_ps = {}
                for h in (2 * t, 2 * t + 1):
                    ctx_ps[h] = pcx.tile([DH + 1, S], f32, tag="ctx", name=f"ctx{h}")
                for j in range(NS):
                    filler(j)
                    for h in (2 * t, 2 * t + 1):
                        d0 = 64 * (h % 2)
                        dsl = slice(d0, d0 + 64)
                        sc = psc.tile([128, S], f32, tag="sc")
                        for c in range(S // QS):
                            cs = slice(c * QS, (c + 1) * QS)
                            nc.tensor.matmul(
                                sc[:, cs],
                                kT[dsl, t, j * 128 : (j + 1) * 128],
                                qT[dsl, t, cs],
                                start=True,
                                stop=True,
                            )
                        et = exps.tile([128, S], f32r, tag="exp")
                        nc.scalar.activation(et, sc, Exp)
                        if debug and h == 0 and j == 0:
                            nc.sync.dma_start(out=exp_dbg[:], in_=et.bitcast(f32))
                        va = v[:, j, :].rearrange("p (h e) -> p h e", e=DH + 1)[:, h, :]
                        for c in range(S // QS):
                            cs = slice(c * QS, (c + 1) * QS)
                            nc.tensor.matmul(
                                ctx_ps[h][:, cs],
                                va,
                                et[:, cs],
                                start=(j == 0),
                                stop=(j == NS - 1),
                            )
                for h in (2 * t, 2 * t + 1):
                    cs_sb = fin.tile([DH + 1, S], f32, tag="ctx_sb")
                    nc.vector.tensor_copy(cs_sb, ctx_ps[h])
                    if debug and h == 0:
                        nc.sync.dma_start(out=ctxraw_dbg[:], in_=cs_sb)
                    rec = fin.tile([1, S], f32r, tag="rec")
                    with nc.allow_low_precision(reason="f32r round for bcast"):
                        nc.vector.reciprocal(rec, cs_sb[DH : DH + 1, :])
                    bc = pcx.tile([DH, S], f32, tag="ctx", name=f"bc{h}")
                    for c in range(S // QS):
                        cs = slice(c * QS, (c + 1) * QS)
                        nc.tensor.matmul(
                            bc[:, cs], ones64, rec[:, cs],
                            start=True, stop=True,
                        )
                    nc.vector.tensor_tensor(
                        out=cs_sb[0:DH, :],
                        in0=cs_sb[0:DH, :],
                        in1=bc,
                        op=mult,
                    )
                    nc.sync.dma_start(out=out_d[h], in_=cs_sb[0:DH, :])

            # ---- emission schedule ----
            def mk(w, dest, bias, c, t):
                def emit():
                    cs = slice(c * QS, (c + 1) * QS)
                    ps = pp.tile([128, QS], f32, tag="proj")
                    for k in range(NT):
                        nc.tensor.matmul(
                            ps,
                            wT[w][:, k, t * 128 : (t + 1) * 128],
                            hsT[:, k, cs],
                            start=(k == 0),
                            stop=(k == NT - 1),
                        )
                    nc.vector.tensor_scalar_add(
                        dest[:, t, cs], ps, bias[:, t : t + 1]
                    )
                    if w == "k":
                        nc.vector.tensor_tensor(
                            out=dest[:, t, cs],
                            in0=dest[:, t, cs],
                            in1=clickB[:, cs],
                            op=mult,
                        )

                return emit

            def qk_chunks(t):
                return [
                    mk(w, dest, bias, c, t)
                    for w, dest, bias in (("q", qT, bqT), ("k", kT, bkT))
                    for c in range(S // QS)
                ]

            proj_qk(0)
            t1_chunks = qk_chunks(1)

            def fill_v(j):
                proj_v(j)
                if j >= NS - len(t1_chunks):
                    t1_chunks[j - (NS - len(t1_chunks))]()

            attn_pair(0, fill_v)

            for t in range(1, NT):
                chunks = []
                if t < NT - 1:
                    chunks = qk_chunks(t + 1)

                def fill(j, chunks=chunks):
                    if j < len(chunks):
                        chunks[j]()

                attn_pair(t, fill)

            if debug:
                nc.sync.dma_start(out=qT_dbg[:], in_=qT.bitcast(f32))
                nc.sync.dma_start(out=kT_dbg[:], in_=kT.bitcast(f32))
                nc.sync.dma_start(out=v_dbg[:], in_=v.bitcast(f32))

    _install_multiwait_split(nc)
    return nc


def _install_multiwait_split(nc):
    """This walrus build accepts at most one sync wait per instruction
    (Drain/CTRL and Matmult/LDWEIGHTS structs at least). Tile attaches
    several. Split extras onto single-wait NoOps inserted just before the
    instruction, at JSON-serialization time so every compile path sees it."""
    import types

    import orjson
    from concourse import mybir

    def to_json_bytes(self):
        m = orjson.loads(mybir.module_to_json_bytes(self.m))
        n = 0
        for fn in m.get("functions", []):
            for bb in fn.get("blocks", []):
                insts = bb.get("instructions", [])
                out = []
                for inst in insts:
                    si = inst.get("sync_info")
                    waits = (si or {}).get("on_wait") or []
                    if len(waits) > 1:
                        for w in waits[:-1]:
                            n += 1
                            out.append(
                                {
                                    "debug": inst.get("debug", 0),
                                    "engine": inst["engine"],
                                    "ins": [],
                                    "name": f"I-mws{n}",
                                    "opcode": "NoOp",
                                    "outs": [],
                                    "sync_info": {"on_update": [], "on_wait": [w]},
                                    "text_hint": "multiwait_split",
                                }
                            )
                        si["on_wait"] = [waits[-1]]
                    out.append(inst)
                bb["instructions"] = out
        return orjson.dumps(m)

    nc.to_json_bytes = types.MethodType(to_json_bytes, nc)


def _get_built():
    global _built
    if _built is None:
        _apply_workarounds()
        _built = _build()
    return _built


def _prep_in_maps(inputs):
    hs = np.asarray(inputs["hidden_states"], np.float32)
    mask = np.asarray(inputs["attention_mask"], np.float32)
    click = np.asarray(inputs["click_times"], np.float32)
    Wq = np.asarray(inputs["Wq"], np.float32)
    bq = np.asarray(inputs["bq"], np.float32)
    Wk = np.asarray(inputs["Wk"], np.float32)
    bk = np.asarray(inputs["bk"], np.float32)
    Wv = np.asarray(inputs["Wv"], np.float32)
    bv = np.asarray(inputs["bv"], np.float32)

    scale = 1.0 / np.sqrt(np.float32(DH))
    shared = {
        "wqT": np.ascontiguousarray(Wq.T),
        "wkT": np.ascontiguousarray(Wk.T * scale),
        "wvT": np.ascontiguousarray(Wv.T),
        "bqT": np.ascontiguousarray(bq.reshape(NT, 128).T),
        "bkT": np.ascontiguousarray((bk * scale).reshape(NT, 128).T),
        "bvB": np.ascontiguousarray(np.broadcast_to(bv, (128, D))),
        "ones64": np.ones((1, DH), np.float32),
        "vones": np.ones((128, NS, H), np.float32),
    }
    in_maps = []
    for b in range(B):
        m = dict(shared)
        m["hsT"] = np.ascontiguousarray(hs[b].T)
        m["clickB"] = np.ascontiguousarray(np.broadcast_to(click[b], (128, S)))
        m["maskT"] = np.ascontiguousarray(
            np.exp(mask[b, 0, 0].astype(np.float64)).astype(np.float32).reshape(NS, 128).T
        )
        in_maps.append(m)
    return in_maps


def run(inputs, trace=False, tmpdir=None):
    """Run on the 8 cores; returns (output [B,S,D], BassKernelResults)."""
    from concourse.bass_utils import run_bass_kernel_spmd

    nc = _get_built()
    in_maps = _prep_in_maps(inputs)
    res = run_bass_kernel_spmd(
        nc, in_maps, list(range(B)), trace=trace, tmpdir=tmpdir
    )
    out = np.empty((B, S, D), np.float32)
    for b in range(B):
        ctxT = res.results[b]["out"]  # [H, DH, S]
        out[b] = ctxT.transpose(2, 0, 1).reshape(S, D)
    return out, res


def kernel(**inputs) -> np.ndarray:
    out, _ = run(inputs)
    return out


# revision 16
# speedup vs baseline: 1.0438x; 1.0438x over previous
"""BERT self-attention (B=8, S=1024, D=768, H=12) on 8 TRN2 NeuronCores.

Sharding: batch across the 8 cores (one batch element per core).

Per-core dataflow (all matmuls fp32r on the tensor engine):
  - host pre-transposes hs[b] -> hsT [D, S] and the weights -> W.T [D, D]
    so the contraction dim (din) lands on SBUF partitions.
  - qT[dout, s], k'T[dout, s] = W.T-tiles (stationary) x hsT (moving);
    k'T additionally folds the 1/sqrt(dh) scale (host, into Wk/bk) and the
    multiplicative click_times[ks] gate (on-chip, per-column multiply).
  - v[s, dout] = hsT-tiles (stationary) x Wv.T (moving), stored head-major
    [s, (h, 65)] with a ones column per head; rows scaled by exp(mask[ks])
    (folds the additive attention mask through the exp).
  - per head: scoresT[ks, qs] = kT_head.T @ qT_head (K=64; adjacent heads
    packed concurrently via PE row tiling), expT = Exp(scoresT) straight
    out of PSUM on the scalar engine, ctxT[65, qs] accumulates
    v_aug.T @ expT over ks; row 64 is the softmax denominator.
  - normalize: partition-broadcast of the denominator row + reciprocal +
    multiply; DMA ctxT per head; host transposes back on unshard.
"""

import sys

sys.path.insert(0, "/opt/trn_rl_repo")

import numpy as np

B, S, D, H = 8, 1024, 768, 12
DH = D // H  # 64
NT = D // 128  # 6 dout/din tiles
NS = S // 128  # 8 s tiles
QS = 512  # qs chunk (fp32 moving-operand max)

_built = None


def _apply_workarounds():
    """Container fixes: (1) walrus here accepts at most one sync wait on the
    Tile tail Drain -> split extra waits onto SP nops; (2) antenv.axon_hooks
    is missing from the image (needed only for trace=True profiling)."""
    import os

    import concourse.tile as tile
    from concourse.vector_clock import ScopedClock

    if getattr(tile.TileContext, "_drain_split_patched", False):
        return

    def _drain_and_barrier(self, tick_clock, wait_clock):
        drain_inst = self.nc.sync.drain()
        wait_clock.add_sem_waits(
            drain_inst.ins, ScopedClock({None: tick_clock.global_clock})
        )
        si = drain_inst.ins.sync_info
        if si is not None and len(si.on_wait) > 1:
            waits = list(si.on_wait)
            si.on_wait = waits[:1]
            for w in waits[1:]:
                nop = self.nc.sync.nop(nofuse=True, hint="drain_wait_split")
                nsi = nop.ins.sync_info
                if nsi is None:
                    import bass_rust

                    nop.ins.sync_info = bass_rust.SyncInfo(on_update=[], on_wait=[w])
                else:
                    nsi.on_wait = [w]

        self.nc.all_engine_barrier()
        assert self.sems is not None
        popped = self.nc._tile_sem_poison_stack.pop()
        assert popped is self._sem_poison
        self.nc.clear_and_free_semaphores(list(self.sems.allocated().values()))
        self.nc.all_engine_barrier()

    tile.TileContext._drain_and_barrier = _drain_and_barrier
    tile.TileContext._drain_split_patched = True

    hooks_src = (
        "_axon_ntff_profile_hook = None\n\n\n"
        "def set_axon_ntff_profile_hook(hook):\n"
        "    global _axon_ntff_profile_hook\n"
        "    _axon_ntff_profile_hook = hook\n\n\n"
        "def get_axon_ntff_profile_hook():\n"
        "    return _axon_ntff_profile_hook\n"
    )
    for d in ("/root/.axon_site/_ro/trn_rl_repo/antenv", "/opt/trn_rl_repo/antenv"):
        path = os.path.join(d, "axon_hooks.py")
        try:
            if os.path.isdir(d) and not os.path.exists(path):
                with open(path, "w") as f:
                    f.write(hooks_src)
        except OSError:
            pass


def _build(debug=False):
    import concourse.bass as bass
    import concourse.tile as tile
    from concourse import mybir

    f32 = mybir.dt.float32
    f32r = mybir.dt.float32r
    Exp = mybir.ActivationFunctionType.Exp
    mult = mybir.AluOpType.mult

    nc = bass.Bass()
    hsT_d = nc.dram_tensor("hsT", [D, S], f32r, kind="ExternalInput")
    wT_d = {
        w: nc.dram_tensor(f"w{w}T", [D, D], f32r, kind="ExternalInput")
        for w in ("q", "k", "v")
    }
    bqT_d = nc.dram_tensor("bqT", [128, NT], f32, kind="ExternalInput")
    bkT_d = nc.dram_tensor("bkT", [128, NT], f32, kind="ExternalInput")
    bvB_d = nc.dram_tensor("bvB", [128, D], f32, kind="ExternalInput")
    clickB_d = nc.dram_tensor("clickB", [128, S], f32, kind="ExternalInput")
    maskT_d = nc.dram_tensor("maskT", [128, NS], f32, kind="ExternalInput")
    ones64_d = nc.dram_tensor("ones64", [1, DH], f32r, kind="ExternalInput")
    vones_d = nc.dram_tensor("vones", [128, NS, H], f32r, kind="ExternalInput")
    out_d = nc.dram_tensor("out", [H, DH, S], f32, kind="ExternalOutput")
    if debug:
        qT_dbg = nc.dram_tensor("qTd", [128, NT, S], f32, kind="ExternalOutput")
        kT_dbg = nc.dram_tensor("kTd", [128, NT, S], f32, kind="ExternalOutput")
        v_dbg = nc.dram_tensor("vd", [128, NS, H * (DH + 1)], f32, kind="ExternalOutput")
        exp_dbg = nc.dram_tensor("expd", [128, S], f32, kind="ExternalOutput")
        ctxraw_dbg = nc.dram_tensor("ctxrawd", [DH + 1, S], f32, kind="ExternalOutput")

    with tile.TileContext(nc) as tc:
        from contextlib import ExitStack

        with ExitStack() as ctx:
            consts = ctx.enter_context(tc.tile_pool(name="consts", bufs=1))
            big = ctx.enter_context(tc.tile_pool(name="big", bufs=1))
            exps = ctx.enter_context(tc.tile_pool(name="exps", bufs=3))
            fin = ctx.enter_context(tc.tile_pool(name="fin", bufs=2))
            pp = ctx.enter_context(tc.tile_pool(name="pp", bufs=2, space="PSUM"))
            psc = ctx.enter_context(tc.tile_pool(name="psc", bufs=2, space="PSUM"))
            pcx = ctx.enter_context(tc.tile_pool(name="pcx", bufs=1, space="PSUM"))

            # ---- constants ----
            bqT = consts.tile([128, NT], f32)
            nc.sync.dma_start(out=bqT, in_=bqT_d[:])
            bkT = consts.tile([128, NT], f32)
            nc.sync.dma_start(out=bkT, in_=bkT_d[:])
            bvB = consts.tile([128, D], f32)
            nc.sync.dma_start(out=bvB, in_=bvB_d[:])
            clickB = consts.tile([128, S], f32)
            nc.sync.dma_start(out=clickB, in_=clickB_d[:])
            maskT = consts.tile([128, NS], f32)
            nc.sync.dma_start(out=maskT, in_=maskT_d[:])
            ones64 = consts.tile([1, DH], f32r)
            nc.sync.dma_start(out=ones64, in_=ones64_d[:])

            # ---- inputs, split per k-tile so the first matmuls start early ----
            hsT = big.tile([128, NT, S], f32r)
            wT = {}
            for w in ("q", "k", "v"):
                wT[w] = big.tile([128, NT, D], f32r, tag=f"w{w}", name=f"w{w}sb")
            hsT_r = hsT_d.rearrange("(t p) s -> p t s", p=128)
            wT_r = {w: wT_d[w].rearrange("(t p) d -> p t d", p=128) for w in wT_d}
            for k in range(NT):
                nc.sync.dma_start(out=hsT[:, k, :], in_=hsT_r[:, k, :])
                nc.sync.dma_start(out=wT["q"][:, k, :], in_=wT_r["q"][:, k, :])
            for w in ("k", "v"):
                for k in range(NT):
                    nc.sync.dma_start(out=wT[w][:, k, :], in_=wT_r[w][:, k, :])

            qT = big.tile([128, NT, S], f32r, tag="qT")
            kT = big.tile([128, NT, S], f32r, tag="kT")
            # v_aug: [s_partition, s_tile, head-major (h, dh | ones)]
            v = big.tile([128, NS, H * (DH + 1)], f32r, tag="v")

            def qk_chunk(w, dest, bias, c, t):
                cs = slice(c * QS, (c + 1) * QS)
                ps = pp.tile([128, QS], f32, tag="proj")
                for k in range(NT):
                    nc.tensor.matmul(
                        ps,
                        wT[w][:, k, t * 128 : (t + 1) * 128],
                        hsT[:, k, cs],
                        start=(k == 0),
                        stop=(k == NT - 1),
                    )
                nc.vector.tensor_scalar_add(dest[:, t, cs], ps, bias[:, t : t + 1])
                if w == "k":
                    nc.vector.tensor_tensor(
                        out=dest[:, t, cs],
                        in0=dest[:, t, cs],
                        in1=clickB[:, cs],
                        op=mult,
                    )

            def qk_chunks(t):
                return [
                    (lambda w=w, dest=dest, bias=bias, c=c: qk_chunk(w, dest, bias, c, t))
                    for w, dest, bias in (("q", qT, bqT), ("k", kT, bkT))
                    for c in range(S // QS)
                ]

            def proj_v(si):
                """v rows for s-tile si, head-major with ones col, mask-scaled."""
                vsi = v[:, si, :].rearrange("p (h e) -> p h e", e=DH + 1)
                for c0, cn in ((0, 512), (512, 256)):
                    h0, nh = c0 // DH, cn // DH
                    ps = pp.tile([128, cn], f32, tag="proj")
                    for k in range(NT):
                        nc.tensor.matmul(
                            ps,
                            hsT[:, k, si * 128 : (si + 1) * 128],
                            wT["v"][:, k, c0 : c0 + cn],
                            start=(k == 0),
                            stop=(k == NT - 1),
                        )
                    nc.vector.tensor_tensor(
                        out=vsi[:, h0 : h0 + nh, 0:DH],
                        in0=ps.rearrange("p (h e) -> p h e", e=DH),
                        in1=bvB[:, c0 : c0 + cn].rearrange("p (h e) -> p h e", e=DH),
                        op=mybir.AluOpType.add,
                    )
                nc.sync.dma_start(out=vsi[:, :, DH : DH + 1], in_=vones_d[:, si, :])
                nc.vector.tensor_scalar_mul(
                    v[:, si, :], v[:, si, :], maskT[:, si : si + 1]
                )

            def attn_head(h, filler):
                """One head; filler(j) emits PE gap-filling projection work."""
                t, d0 = h // 2, 64 * (h % 2)
                dsl = slice(d0, d0 + 64)
                ctx_ps = pcx.tile([DH + 1, S], f32, tag="ctx", name=f"ctx{h}")
                for j in range(NS):
                    filler(j)
                    sc = psc.tile([128, S], f32, tag="sc")
                    for c in range(S // QS):
                        cs = slice(c * QS, (c + 1) * QS)
                        nc.tensor.matmul(
                            sc[:, cs],
                            kT[dsl, t, j * 128 : (j + 1) * 128],
                            qT[dsl, t, cs],
                            start=True,
                            stop=True,
                        )
                    et = exps.tile([128, S], f32r, tag="exp")
                    nc.scalar.activation(et, sc, Exp)
                    if debug and h == 0 and j == 0:
                        nc.sync.dma_start(out=exp_dbg[:], in_=et.bitcast(f32))
                    va = v[:, j, :].rearrange("p (h e) -> p h e", e=DH + 1)[:, h, :]
                    for c in range(S // QS):
                        cs = slice(c * QS, (c + 1) * QS)
                        nc.tensor.matmul(
                            ctx_ps[:, cs],
                            va,
                            et[:, cs],
                            start=(j == 0),
                            stop=(j == NS - 1),
                        )
                cs_sb = fin.tile([DH + 1, S], f32, tag="ctx_sb")
                nc.vector.tensor_copy(cs_sb, ctx_ps)
                if debug and h == 0:
                    nc.sync.dma_start(out=ctxraw_dbg[:], in_=cs_sb)
                rec = fin.tile([1, S], f32r, tag="rec")
                with nc.allow_low_precision(reason="f32r round for bcast"):
                    nc.vector.reciprocal(rec, cs_sb[DH : DH + 1, :])
                bc = psc.tile([DH, S], f32, tag="sc", name=f"bc{h}")
                for c in range(S // QS):
                    cs = slice(c * QS, (c + 1) * QS)
                    nc.tensor.matmul(
                        bc[:, cs], ones64, rec[:, cs], start=True, stop=True
                    )
                nc.vector.tensor_tensor(
                    out=cs_sb[0:DH, :], in0=cs_sb[0:DH, :], in1=bc, op=mult
                )
                nc.sync.dma_start(out=out_d[h], in_=cs_sb[0:DH, :])

            # ---- emission schedule ----
            # qk tile 0 up front; v folded into head 0's loop; qk tile t+1
            # emitted as fillers during heads 2t, 2t+1.
            for fn in qk_chunks(0):
                fn()

            t1 = qk_chunks(1)
            attn_head(0, lambda j: proj_v(j))
            attn_head(1, lambda j: t1[j // 2]() if j % 2 == 0 else None)

            tile_chunks = {t: qk_chunks(t + 1) for t in range(1, NT - 1)}
            for h in range(2, H):
                t = h // 2
                chunks = tile_chunks.get(t, [])
                # 4 chunks per tile t+1, 2 per head: even head 0,1; odd 2,3
                sel = chunks[0:2] if h % 2 == 0 else chunks[2:4]

                def fill(j, sel=sel):
                    if j == 1 and len(sel) > 0:
                        sel[0]()
                    elif j == 4 and len(sel) > 1:
                        sel[1]()

                attn_head(h, fill)

            if debug:
                nc.sync.dma_start(out=qT_dbg[:], in_=qT.bitcast(f32))
                nc.sync.dma_start(out=kT_dbg[:], in_=kT.bitcast(f32))
                nc.sync.dma_start(out=v_dbg[:], in_=v.bitcast(f32))

    _install_multiwait_split(nc)
    return nc


def _install_multiwait_split(nc):
    """This walrus build accepts at most one sync wait per instruction
    (Drain/CTRL and Matmult/LDWEIGHTS structs at least). Tile attaches
    several. Split extras onto single-wait NoOps inserted just before the
    instruction, at JSON-serialization time so every compile path sees it."""
    import types

    import orjson
    from concourse import mybir

    def to_json_bytes(self):
        m = orjson.loads(mybir.module_to_json_bytes(self.m))
        n = 0
        for fn in m.get("functions", []):
            for bb in fn.get("blocks", []):
                insts = bb.get("instructions", [])
                out = []
                for inst in insts:
                    si = inst.get("sync_info")
                    waits = (si or {}).get("on_wait") or []
                    if len(waits) > 1:
                        for w in waits[:-1]:
                            n += 1
                            out.append(
                                {
                                    "debug": inst.get("debug", 0),
                                    "engine": inst["engine"],
                                    "ins": [],
                                    "name": f"I-mws{n}",
                                    "opcode": "NoOp",
                                    "outs": [],
                                    "sync_info": {"on_update": [], "on_wait": [w]},
                                    "text_hint": "multiwait_split",
                                }
                            )
                        si["on_wait"] = [waits[-1]]
                    out.append(inst)
                bb["instructions"] = out
        return orjson.dumps(m)

    nc.to_json_bytes = types.MethodType(to_json_bytes, nc)


def _get_built():
    global _built
    if _built is None:
        _apply_workarounds()
        _built = _build()
    return _built


def _prep_in_maps(inputs):
    hs = np.asarray(inputs["hidden_states"], np.float32)
    mask = np.asarray(inputs["attention_mask"], np.float32)
    click = np.asarray(inputs["click_times"], np.float32)
    Wq = np.asarray(inputs["Wq"], np.float32)
    bq = np.asarray(inputs["bq"], np.float32)
    Wk = np.asarray(inputs["Wk"], np.float32)
    bk = np.asarray(inputs["bk"], np.float32)
    Wv = np.asarray(inputs["Wv"], np.float32)
    bv = np.asarray(inputs["bv"], np.float32)

    scale = 1.0 / np.sqrt(np.float32(DH))
    shared = {
        "wqT": np.ascontiguousarray(Wq.T),
        "wkT": np.ascontiguousarray(Wk.T * scale),
        "wvT": np.ascontiguousarray(Wv.T),
        "bqT": np.ascontiguousarray(bq.reshape(NT, 128).T),
        "bkT": np.ascontiguousarray((bk * scale).reshape(NT, 128).T),
        "bvB": np.ascontiguousarray(np.broadcast_to(bv, (128, D))),
        "ones64": np.ones((1, DH), np.float32),
        "vones": np.ones((128, NS, H), np.float32),
    }
    in_maps = []
    for b in range(B):
        m = dict(shared)
        m["hsT"] = np.ascontiguousarray(hs[b].T)
        m["clickB"] = np.ascontiguousarray(np.broadcast_to(click[b], (128, S)))
        m["maskT"] = np.ascontiguousarray(
            np.exp(mask[b, 0, 0].astype(np.float64)).astype(np.float32).reshape(NS, 128).T
        )
        in_maps.append(m)
    return in_maps


def run(inputs, trace=False, tmpdir=None):
    """Run on the 8 cores; returns (output [B,S,D], BassKernelResults)."""
    from concourse.bass_utils import run_bass_kernel_spmd

    nc = _get_built()
    in_maps = _prep_in_maps(inputs)
    res = run_bass_kernel_spmd(
        nc, in_maps, list(range(B)), trace=trace, tmpdir=tmpdir
    )
    out = np.empty((B, S, D), np.float32)
    for b in range(B):
        ctxT = res.results[b]["out"]  # [H, DH, S]
        out[b] = ctxT.transpose(2, 0, 1).reshape(S, D)
    return out, res


def kernel(**inputs) -> np.ndarray:
    out, _ = run(inputs)
    return out
